# revision 28
# baseline (speedup 1.0000x reference)
"""Multi-head attention TRN2 Bass kernel (v2).

Problem: B=4, N=2048, D=E=512, 8 heads (ch=64).
out = softmax((x_q Wq + bq)(x_k Wk + bk)^T / 8) (x_v Wv + bv), per head.

Sharding (8 cores): core c handles batch b = c//2 and head-group g = c%2
(4 heads = 256 E-columns). Each core is fully independent (no collectives).

v2 changes over the original ACT-paced design:
  - Pass = (head-pair, i-chunk of 512). The two heads of a pair occupy
    SBUF partitions 0-63 / 64-127 of QT/KT, so their S^T matmuls issue as
    back-to-back row-tiled pairs (tile_position (0,0)/(64,0)) that execute
    CONCURRENTLY on the PE (HW-probed: 113 ns/MM vs 215 serial, 1.9x).
  - Part of the exp work moves off the ACT engine onto the DVE as a
    Schraudolph bit-trick: P_bf16bits = int16(rint(A*S + B)), one
    tensor_scalar (fp32 PSUM -> int16 SBUF, round-to-nearest verified on
    HW), bitcast to bf16 for the AV matmul. Host-simulated rel-err with
    this split: ~0.012 (gate 0.02).
  - Inputs are host-interleaved so every chunk DMA is a flat 2D copy with
    4KB-contiguous lines per partition (DMA packets are overhead-bound);
    biases ride one [128,4] tensor. Later waves are WAW-gated behind the
    critical Q/K pieces via tiny pre-writes into their destinations.
  - PE warm-up dummies + early exp-table preload hide the HAM cold clock
    (4/8 = 1.2 GHz) and the 2.7us ACT table load during the input DMA.
  - Output blocks are staged 4-at-a-time in SBUF and written with one DMA
    per (head, 512-chunk): 16 output DMAs instead of 64.

v3: output stays transposed. The AV result OT [ch+1, i] is divided by the
den row in-place ([ch, i] layout) and DMA'd per pass; the host reassembly
transposes blocks (it already reshuffles them). Removes all 64 PE
transposes (~10us of PE), the per-block DVE recip/mul path (~25us of
DVE), and shrinks the post-matmul tail. Division chain per pass:
reciprocal_approx_fast on the den row -> gpsimd partition_broadcast ->
one tensor_tensor multiply.
"""

import numpy as np
import ml_dtypes

import concourse.bacc as bacc
import concourse.mybir as mybir
import concourse.tile as tile
from concourse.bass_utils import run_bass_kernel_spmd

B, N, D, E = 4, 2048, 512, 512
H, CH = 8, 64
HPC = 4              # heads per core
EC = HPC * CH        # 256 E-columns per core
SCALE = 1.0 / 8.0    # 1/sqrt(CH)
NT = N // 128        # 16 j-tiles
DT = D // 128        # 4 d-tiles

SIGMA = 0.055
A_SCH = float(np.float32(128.0 * np.log2(np.e) * SCALE))
B_SCH = float(np.float32(128.0 * (127.0 - SIGMA)))
# Schraudolph j-tiles per pass (ACT exp alone paces both pass 0's ramp and
# the steady passes; host err sim: 4 tiles/pass -> 0.0122, 5 -> 0.0128,
# gate 0.02; j=15 keeps the pass-boundary st WAR off the last ACT)
# j=14,15 both on the DVE: the next pass's prologue S-pairs WAR on those
# two st buffers, and an ACT-side release costs ~1us at each pass boundary
SCH_BY_PASS = [(2, 5, 9, 12)] + [(2, 5, 9, 12, 14, 15)] * 7

F32 = mybir.dt.float32
BF16 = mybir.dt.bfloat16
I16 = mybir.dt.int16
NP_BF16 = ml_dtypes.bfloat16

_cache = {}


def _build():
    nc = bacc.Bacc("TRN2", target_bir_lowering=False, debug=False)

    # x tensors host-interleaved: row p = [c-major][t-major][n'] so a
    # 512-column chunk is 4KB contiguous per row (DMA packets are
    # overhead-bound, so line size is the bandwidth lever)
    xq = nc.dram_tensor("xq", [128, DT * N], BF16, kind="ExternalInput")
    xk = nc.dram_tensor("xk", [128, DT * N], BF16, kind="ExternalInput")
    xv = nc.dram_tensor("xv", [128, DT * N], BF16, kind="ExternalInput")
    wq = nc.dram_tensor("wq", [128, DT * EC], BF16, kind="ExternalInput")
    wk = nc.dram_tensor("wk", [128, DT * EC], BF16, kind="ExternalInput")
    wv = nc.dram_tensor("wv", [128, DT * EC], BF16, kind="ExternalInput")
    # all four bias vectors as one [128, 4] tensor (cols: bq m0, bq m1,
    # bk m0, bk m1) -- a [128,1] DMA degenerates to 4-byte packets
    bmat = nc.dram_tensor("bmat", [128, 4], F32, kind="ExternalInput")
    bvr = nc.dram_tensor("bvr", [128, EC], F32, kind="ExternalInput")
    # one [64ch, 1024] block per pass (cols: head-pair x 512 i); host
    # reassembles + transposes
    out = nc.dram_tensor("out", [8 * 64, 1024], F32, kind="ExternalOutput")

    with tile.TileContext(nc) as tc:
        with (
            tc.tile_pool(name="singles", bufs=1) as singles,
            tc.tile_pool(name="qkv", bufs=1) as qkv,
            tc.tile_pool(name="fin", bufs=3) as fin_pool,
        ):
            # ---- SBUF staging ----
            dummy = singles.tile([128, 512], BF16, tag="dummy", name="dummy")
            # flat, chunk-major (c, t, n') so every chunk DMA is a 2D copy
            # with 4KB contiguous per partition (max DMA packet size)
            xq_sb = singles.tile([128, DT * N], BF16, tag="xq", name="xq")
            xk_sb = singles.tile([128, DT * N], BF16, tag="xk", name="xk")
            xv_sb = singles.tile([128, DT * N], BF16, tag="xv", name="xv")
            wq_sb = singles.tile([128, DT * EC], BF16, tag="wq", name="wq")
            wk_sb = singles.tile([128, DT * EC], BF16, tag="wk", name="wk")
            wv_sb = singles.tile([128, DT * EC], BF16, tag="wv", name="wv")
            bm_sb = singles.tile([128, 4], F32, tag="bm", name="bm")
            bq_sb = [bm_sb[:, m:m + 1] for m in range(2)]
            bk_sb = [bm_sb[:, 2 + m:3 + m] for m in range(2)]
            bvr_sb = singles.tile([128, EC], F32, tag="bvr", name="bvr")

            # ---- engine warm-up (emitted first on their queues) ----
            nc.vector.memset(dummy, 0.0)
            gate_sb = singles.tile([1, 8], BF16, tag="gate", name="gate")

            # ---- input DMAs: merged descriptors, critical-first ----
            def xq_c(c):
                return (xq_sb[:, c * 2048:(c + 1) * 2048],
                        xq[:, c * 2048:(c + 1) * 2048])

            def xk_c(c):
                return (xk_sb[:, c * 2048:(c + 1) * 2048],
                        xk[:, c * 2048:(c + 1) * 2048])

            def xv_c(c):
                return (xv_sb[:, c * 2048:(c + 1) * 2048],
                        xv[:, c * 2048:(c + 1) * 2048])

            def cview(sb):  # [128, 4c*4t*512] -> [128, c, t, n']
                return sb.rearrange("p (c t n) -> p c t n", c=4, t=DT)

            def wview(sb):  # [128, 4t*EC] -> [128, t, e]
                return sb.rearrange("p (t e) -> p t e", t=DT)

            # wave 1 (ungated): QK projection critical path + V pass-0 needs.
            # Later waves are WAW-gated by tiny pre-writes into the DMA
            # destinations (emission order alone does not survive the tile
            # scheduler). Gates live on the GPSIMD queue, which is idle
            # during the ramp -- on the vector queue they head-of-line
            # blocked the prologue bias adds for ~8us.
            nc.sync.dma_start(wq_sb, wq[:, :])
            nc.sync.dma_start(*xq_c(0))
            nc.scalar.dma_start(bm_sb, bmat[:, :])
            nc.scalar.dma_start(wk_sb, wk[:, :])
            nc.scalar.dma_start(*xk_c(0))
            nc.gpsimd.dma_start(wv_sb, wv[:, :])
            nc.gpsimd.dma_start(*xv_c(0))
            nc.gpsimd.dma_start(bvr_sb, bvr[:, :])
            # ACT table preload, after scalar's DMA issues
            actwarm = singles.tile([1, 8], BF16, tag="actwarm", name="actwarm")
            nc.scalar.activation(
                actwarm, dummy[0:1, 0:8], mybir.ActivationFunctionType.Exp,
                scale=SCALE,
            )
            # wave 2: gated on xk c0 arrival
            nc.gpsimd.tensor_copy(xk_sb[0:1, 2048:2050], xk_sb[0:1, 0:2])
            nc.gpsimd.dma_start(*xk_c(1))
            # wave 3: gated on xv c0 arrival (last wave-1 x piece), issued
            # in consumption order; each chunk DMA WAW-waits its pre-write
            trig = xv_sb[0:1, 0:2]
            for sb, cc, dm in ((xv_sb, 1, xv_c), (xk_sb, 2, xk_c),
                               (xv_sb, 2, xv_c), (xk_sb, 3, xk_c),
                               (xv_sb, 3, xv_c), (xq_sb, 1, xq_c),
                               (xq_sb, 2, xq_c), (xq_sb, 3, xq_c)):
                nc.gpsimd.tensor_copy(
                    sb[0:1, cc * 2048:cc * 2048 + 2], trig)
                nc.gpsimd.dma_start(*dm(cc))

            # ---- working tiles ----
            qt_sb = [qkv.tile([128, N], BF16, tag=f"qt{m}", name=f"qt{m}") for m in range(2)]
            kt_sb = [qkv.tile([128, N], BF16, tag=f"kt{m}", name=f"kt{m}") for m in range(2)]
            # ones column FIRST so den lands at psum partition 0 (custom DVE
            # psum reads drop partition offsets on HW; keep all reads at 0)
            v_sb = [qkv.tile([128, HPC * 65], BF16, tag=f"v{t}", name=f"v{t}") for t in range(NT)]
            for t in range(NT):
                ones_view = v_sb[t].rearrange("p (h c) -> p h c", c=65)[:, :, 0:1]
                nc.vector.memset(ones_view, 1.0)


            with (
                tc.tile_pool(name="proj_ps", bufs=2, space="PSUM") as proj_ps,
                tc.tile_pool(name="st_ps", bufs=2, space="PSUM") as st_ps,
                tc.tile_pool(name="ot_ps", bufs=1, space="PSUM") as ot_ps,
                tc.tile_pool(name="pt_sb", bufs=6) as pt_pool,
                tc.tile_pool(name="pti_sb", bufs=4) as pti_pool,
            ):
                # PE warm-up: dummy matmuls flip HAM to 8/8 during DMA (the
                # DMA engines clock with HAM: ramp bursts measured 113-139
                # GB/s during mixed-clock windows vs 266-281 GB/s at 8/8)
                for i in range(17):
                    ps = proj_ps.tile([128, 512], F32, tag="proj", name="warm")
                    nc.tensor.matmul(ps, lhsT=dummy[:, 0:128], rhs=dummy,
                                     start=True, stop=True)

                # -- emitters --
                def emit_qk_group(dst, w_s, x_s, b_s, hp, nch):
                    ps = proj_ps.tile([128, 512], F32, tag="proj", name="qkp")
                    for t in range(DT):
                        nc.tensor.matmul(
                            ps,
                            lhsT=wview(w_s)[:, t, hp * 128:(hp + 1) * 128],
                            rhs=cview(x_s)[:, nch, t, :],
                            start=(t == 0),
                            stop=(t == DT - 1),
                        )
                    nc.vector.tensor_scalar_add(
                        dst[hp][:, nch * 512:(nch + 1) * 512], ps, b_s[hp]
                    )

                def emit_v_group(t):
                    ps = proj_ps.tile([128, EC], F32, tag="proj", name="vp")
                    for d in range(DT):
                        nc.tensor.matmul(
                            ps,
                            lhsT=cview(xv_sb)[:, t // 4, d,
                                              (t % 4) * 128:(t % 4 + 1) * 128],
                            rhs=wview(wv_sb)[:, d, :],
                            start=(d == 0),
                            stop=(d == DT - 1),
                        )
                    v_view = v_sb[t].rearrange("p (h c) -> p h c", c=65)[:, :, 1:65]
                    nc.vector.tensor_add(
                        v_view,
                        ps.rearrange("p (h c) -> p h c", c=64),
                        bvr_sb.rearrange("p (h c) -> p h c", c=64),
                    )

                def emit_filler(f):
                    if f[0] == "v":
                        emit_v_group(f[1])
                    elif f[0] == "q":
                        emit_qk_group(qt_sb, wq_sb, xq_sb, bq_sb, f[1], f[2])
                    else:
                        emit_qk_group(kt_sb, wk_sb, xk_sb, bk_sb, f[1], f[2])

                def emit_s_pair(hp, icol, j):
                    st = st_ps.tile([128, 1024], F32, tag="st", name="st")
                    for half in range(2):
                        ho = half * 64
                        nc.tensor.matmul(
                            st[:, half * 512:(half + 1) * 512],
                            lhsT=kt_sb[hp][ho:ho + 64, j * 128:(j + 1) * 128],
                            rhs=qt_sb[hp][ho:ho + 64, icol:icol + 512],
                            start=True,
                            stop=True,
                        )
                    return st

                def emit_out_pass(p, ot):
                    # divide OT by its den row (psum partition 0; custom DVE
                    # psum reads only work offset-0 on HW; DVE partition
                    # ranges must be 32-aligned so the multiply covers all 65
                    # rows and the DMA slices off row 0).
                    if p < 7:
                        # one fast copy frees the ot psum bank for the next
                        # pass (WAR on its first AV); the divide chain then
                        # runs off SBUF at leisure: recip(den) -> partition
                        # broadcast (pool) -> one DVE multiply -> one DMA
                        # copy on the scalar engine: DVE queue latency gates
                        # the Schraudolph st releases; ACT has slack here
                        ots = fin_pool.tile([65, 1024], F32, tag="ots", name="ots")
                        nc.scalar.copy(ots, ot)
                        rec = fin_pool.tile([1, 1024], F32, tag="rec", name="rec")
                        nc.vector.reciprocal_approx_fast(rec, ots[0:1, :])
                        bc = fin_pool.tile([65, 1024], F32, tag="bc", name="bc")
                        nc.gpsimd.partition_broadcast(bc, rec)
                        # multiply on gpsimd (all operands SBUF): keeps the
                        # DVE queue short so Schraudolph st releases stay fast
                        osb = fin_pool.tile([65, 1024], F32, tag="osb", name="osb")
                        nc.gpsimd.tensor_tensor(
                            osb, ots, bc, mybir.AluOpType.mult
                        )
                        eng = nc.sync if p % 2 == 0 else nc.gpsimd
                        eng.dma_start(out[p * 64:(p + 1) * 64, :], osb[1:65, :])
                    else:
                        # last pass: the chain is the kernel tail. Skip the
                        # copy (no next pass -> no WAR) and pipeline two
                        # halves across DVE/pool/DMA to cut exposed latency.
                        for half in range(2):
                            sl = slice(half * 512, (half + 1) * 512)
                            rec = fin_pool.tile([1, 512], F32,
                                                tag=f"rech{half}", name="rec")
                            nc.vector.reciprocal_approx_fast(rec, ot[0:1, sl])
                            bc = fin_pool.tile([65, 512], F32,
                                               tag=f"bch{half}", name="bc")
                            nc.gpsimd.partition_broadcast(bc, rec)
                            osb = fin_pool.tile([65, 512], F32,
                                                tag=f"osbh{half}", name="osb")
                            nc.vector.tensor_tensor(
                                osb, ot[:, sl], bc, mybir.AluOpType.mult
                            )
                            eng = nc.sync if half == 0 else nc.gpsimd
                            eng.dma_start(out[p * 64:(p + 1) * 64, sl],
                                          osb[1:65, :])

                # filler schedule: (pass, iter) -> list of jobs
                fillers = {
                    (0, 0): [("v", 0)], (0, 1): [("k", 0, 1), ("v", 1)],
                    (0, 2): [("v", 2), ("v", 3)], (0, 3): [("v", 4)],
                    (0, 4): [("v", 5), ("v", 6)], (0, 5): [("k", 0, 2)],
                    (0, 6): [("v", 7), ("v", 8)], (0, 7): [("v", 9)],
                    (0, 8): [("v", 10), ("v", 11)], (0, 9): [("k", 0, 3)],
                    (0, 10): [("v", 12), ("v", 13)],
                    (0, 11): [("q", 0, 1)],
                    (0, 12): [("v", 14)], (0, 13): [("v", 15)],
                    (1, 7): [("q", 0, 2)], (1, 13): [("k", 1, 0)],
                    (1, 14): [("k", 1, 1)],
                    (2, 7): [("q", 0, 3)], (2, 13): [("k", 1, 2)],
                    (2, 14): [("k", 1, 3)],
                    (3, 7): [("q", 1, 0)],
                    (4, 7): [("q", 1, 1)],
                    (5, 7): [("q", 1, 2)],
                    (6, 7): [("q", 1, 3)],
                }
                # -- prologue for pass 0 --
                emit_qk_group(qt_sb, wq_sb, xq_sb, bq_sb, 0, 0)
                emit_qk_group(kt_sb, wk_sb, xk_sb, bk_sb, 0, 0)

                # keep the PE (and so HAM) busy through the input-DMA
                # trickle between the prologue projections and the first S
                # pairs; these run in the xk-c0 wait shadow, in the ot psum
                # bank which nothing touches until pass 0's first AV
                for i in range(12):
                    ps = ot_ps.tile([65, 1024], F32, tag="ot", name="warm2")
                    nc.tensor.matmul(ps[:, 0:512], lhsT=dummy[:, 0:65],
                                     rhs=xq_sb[:, 0:512],
                                     start=True, stop=True)

                prologue = [None, None]
                for p in range(8):
                    hp, c = p // 4, p % 4
                    icol = c * 512
                    ha, hb = 2 * hp, 2 * hp + 1
                    sch = SCH_BY_PASS[p]
                    ot = ot_ps.tile([65, 1024], F32, tag="ot", name="ot")
                    sts = [None] * NT
                    pts = [None] * NT

                    if p == 0:
                        sts[0] = emit_s_pair(hp, icol, 0)
                        sts[1] = emit_s_pair(hp, icol, 1)
                    else:
                        sts[0], sts[1] = prologue

                    def emit_av_pair(j):
                        for half, hd in ((0, ha), (1, hb)):
                            nc.tensor.matmul(
                                ot[:, half * 512:(half + 1) * 512],
                                lhsT=v_sb[j][:, hd * 65:(hd + 1) * 65],
                                rhs=pts[j][:, half * 512:(half + 1) * 512],
                                start=(j == 0),
                                stop=(j == NT - 1),
                            )

                    def emit_sch(j):
                        # Schraudolph exp on DVE, one iter ahead of its slot
                        # so the st-buffer WAR never stalls the S pipeline
                        pti = pti_pool.tile([128, 1024], I16, tag="pti", name="pti")
                        nc.vector.tensor_scalar(
                            pti, sts[j], A_SCH, B_SCH,
                            mybir.AluOpType.mult, mybir.AluOpType.add,
                        )
                        pts[j] = pti.bitcast(BF16)

                    if 0 in sch:
                        emit_sch(0)
                    for j in range(NT):
                        if j not in sch:
                            pt = pt_pool.tile([128, 1024], BF16, tag="pt", name="pt")
                            nc.scalar.activation(
                                pt, sts[j], mybir.ActivationFunctionType.Exp,
                                scale=SCALE,
                            )
                            pts[j] = pt
                        if j + 1 in sch:
                            emit_sch(j + 1)
                        if j >= 2:
                            emit_av_pair(j - 2)
                        if j % 2 == 0:
                            for jj in (j + 2, j + 3):
                                if jj < NT:
                                    sts[jj] = emit_s_pair(hp, icol, jj)
                                elif p + 1 < 8:
                                    nhp, nc_ = (p + 1) // 4, (p + 1) % 4
                                    prologue[jj - NT] = emit_s_pair(
                                        nhp, nc_ * 512, jj - NT)
                        for f in fillers.get((p, j), ()):
                            emit_filler(f)

                    emit_av_pair(NT - 2)
                    emit_av_pair(NT - 1)
                    emit_out_pass(p, ot)

    nc.compile()
    return nc


def _get_nc():
    if "nc" not in _cache:
        _cache["nc"] = _build()
    return _cache["nc"]


def _ilv_x(xT):
    # [D, N] -> [128, 4c * 4t * 512n'] with row p = [c][t][n'] interleave
    return np.ascontiguousarray(
        xT.reshape(DT, 128, 4, 512).transpose(1, 2, 0, 3).reshape(128, DT * N)
    ).astype(NP_BF16)


def _ilv_w(w):
    # [D, EC] -> [128, 4t * EC]
    return np.ascontiguousarray(
        w.reshape(DT, 128, EC).transpose(1, 0, 2).reshape(128, DT * EC)
    ).astype(NP_BF16)


def _shard_inputs(q, k, v, Wq, Wk, Wv, bq, bk, bv):
    in_maps = []
    for c in range(8):
        b, g = c // 2, c % 2
        sl = slice(g * EC, (g + 1) * EC)
        in_maps.append({
            "xq": _ilv_x(np.asarray(q)[b].T),
            "xk": _ilv_x(np.asarray(k)[b].T),
            "xv": _ilv_x(np.asarray(v)[b].T),
            "wq": _ilv_w(np.asarray(Wq)[:, sl]),
            "wk": _ilv_w(np.asarray(Wk)[:, sl]),
            "wv": _ilv_w(np.asarray(Wv)[:, sl]),
            "bmat": np.stack([
                np.asarray(bq)[sl][0:128], np.asarray(bq)[sl][128:256],
                np.asarray(bk)[sl][0:128], np.asarray(bk)[sl][128:256],
            ], axis=1).astype(np.float32),
            "bvr": np.ascontiguousarray(
                np.broadcast_to(np.asarray(bv)[sl], (128, EC))
            ).astype(np.float32),
        })
    return in_maps


def kernel(q, k, v, Wq, Wk, Wv, bq, bk, bv, _trace=False):
    nc = _get_nc()
    in_maps = _shard_inputs(q, k, v, Wq, Wk, Wv, bq, bk, bv)
    res = run_bass_kernel_spmd(
        nc, in_maps, core_ids=list(range(8)), trace=_trace
    )
    out = np.empty((B, N, E), np.float32)
    for c in range(8):
        b, g = c // 2, c % 2
        # [8p*64, 1024] -> [hp, cchunk, ch, half, i] -> [N, EC]
        o2 = res.results[c]["out"].reshape(2, 4, 64, 2, 512)
        out[b, :, g * EC:(g + 1) * EC] = (
            o2.transpose(1, 4, 0, 3, 2).reshape(N, EC)
        )
    if _trace:
        _cache["last_exec_time_ns"] = res.exec_time_ns
    return out



# revision 30
# speedup vs baseline: 1.0988x; 1.0988x over previous
"""Multi-head attention TRN2 Bass kernel (v2).

Problem: B=4, N=2048, D=E=512, 8 heads (ch=64).
out = softmax((x_q Wq + bq)(x_k Wk + bk)^T / 8) (x_v Wv + bv), per head.

Sharding (8 cores): core c handles batch b = c//2 and head-group g = c%2
(4 heads = 256 E-columns). Each core is fully independent (no collectives).

v2 changes over the original ACT-paced design:
  - Pass = (head-pair, i-chunk of 512). The two heads of a pair occupy
    SBUF partitions 0-63 / 64-127 of QT/KT, so their S^T matmuls issue as
    back-to-back row-tiled pairs (tile_position (0,0)/(64,0)) that execute
    CONCURRENTLY on the PE (HW-probed: 113 ns/MM vs 215 serial, 1.9x).
  - Part of the exp work moves off the ACT engine onto the DVE as a
    Schraudolph bit-trick: P_bf16bits = int16(rint(A*S + B)), one
    tensor_scalar (fp32 PSUM -> int16 SBUF, round-to-nearest verified on
    HW), bitcast to bf16 for the AV matmul. Host-simulated rel-err with
    this split: ~0.012 (gate 0.02).
  - Inputs are host-interleaved so every chunk DMA is a flat 2D copy with
    4KB-contiguous lines per partition (DMA packets are overhead-bound);
    biases ride one [128,4] tensor. Later waves are WAW-gated behind the
    critical Q/K pieces via tiny pre-writes into their destinations.
  - PE warm-up dummies + early exp-table preload hide the HAM cold clock
    (4/8 = 1.2 GHz) and the 2.7us ACT table load during the input DMA.
  - Output blocks are staged 4-at-a-time in SBUF and written with one DMA
    per (head, 512-chunk): 16 output DMAs instead of 64.

v3: output stays transposed. The AV result OT [ch+1, i] is divided by the
den row in-place ([ch, i] layout) and DMA'd per pass; the host reassembly
transposes blocks (it already reshuffles them). Removes all 64 PE
transposes (~10us of PE), the per-block DVE recip/mul path (~25us of
DVE), and shrinks the post-matmul tail. Division chain per pass:
reciprocal_approx_fast on the den row -> gpsimd partition_broadcast ->
one tensor_tensor multiply.
"""

import numpy as np
import ml_dtypes

import concourse.bacc as bacc
import concourse.mybir as mybir
import concourse.tile as tile
from concourse.bass_utils import run_bass_kernel_spmd

B, N, D, E = 4, 2048, 512, 512
H, CH = 8, 64
HPC = 4              # heads per core
EC = HPC * CH        # 256 E-columns per core
SCALE = 1.0 / 8.0    # 1/sqrt(CH)
NT = N // 128        # 16 j-tiles
DT = D // 128        # 4 d-tiles

SIGMA = 0.055
A_SCH = float(np.float32(128.0 * np.log2(np.e) * SCALE))
B_SCH = float(np.float32(128.0 * (127.0 - SIGMA)))
# Schraudolph j-tiles per pass (ACT exp alone paces both pass 0's ramp and
# the steady passes; host err sim: 4 tiles/pass -> 0.0122, 5 -> 0.0128,
# gate 0.02; j=15 keeps the pass-boundary st WAR off the last ACT)
SCH_BY_PASS = [(2, 5, 9, 12)] + [(2, 5, 9, 12, 15)] * 7

F32 = mybir.dt.float32
BF16 = mybir.dt.bfloat16
I16 = mybir.dt.int16
NP_BF16 = ml_dtypes.bfloat16

_cache = {}


def _build():
    nc = bacc.Bacc("TRN2", target_bir_lowering=False, debug=False)

    # x tensors host-interleaved: row p = [c-major][t-major][n'] so a
    # 512-column chunk is 4KB contiguous per row (DMA packets are
    # overhead-bound, so line size is the bandwidth lever)
    xq = nc.dram_tensor("xq", [128, DT * N], BF16, kind="ExternalInput")
    xk = nc.dram_tensor("xk", [128, DT * N], BF16, kind="ExternalInput")
    xv = nc.dram_tensor("xv", [128, DT * N], BF16, kind="ExternalInput")
    wq = nc.dram_tensor("wq", [128, DT * EC], BF16, kind="ExternalInput")
    wk = nc.dram_tensor("wk", [128, DT * EC], BF16, kind="ExternalInput")
    wv = nc.dram_tensor("wv", [128, DT * EC], BF16, kind="ExternalInput")
    # all four bias vectors as one [128, 4] tensor (cols: bq m0, bq m1,
    # bk m0, bk m1) -- a [128,1] DMA degenerates to 4-byte packets
    bmat = nc.dram_tensor("bmat", [128, 4], F32, kind="ExternalInput")
    bvr = nc.dram_tensor("bvr", [128, EC], F32, kind="ExternalInput")
    # one [64ch, 1024] block per pass (cols: head-pair x 512 i); host
    # reassembles + transposes
    out = nc.dram_tensor("out", [8 * 64, 1024], F32, kind="ExternalOutput")

    with tile.TileContext(nc) as tc:
        with (
            tc.tile_pool(name="singles", bufs=1) as singles,
            tc.tile_pool(name="qkv", bufs=1) as qkv,
            tc.tile_pool(name="fin", bufs=3) as fin_pool,
        ):
            # ---- SBUF staging ----
            dummy = singles.tile([128, 512], BF16, tag="dummy", name="dummy")
            # flat, chunk-major (c, t, n') so every chunk DMA is a 2D copy
            # with 4KB contiguous per partition (max DMA packet size)
            xq_sb = singles.tile([128, DT * N], BF16, tag="xq", name="xq")
            xk_sb = singles.tile([128, DT * N], BF16, tag="xk", name="xk")
            xv_sb = singles.tile([128, DT * N], BF16, tag="xv", name="xv")
            wq_sb = singles.tile([128, DT * EC], BF16, tag="wq", name="wq")
            wk_sb = singles.tile([128, DT * EC], BF16, tag="wk", name="wk")
            wv_sb = singles.tile([128, DT * EC], BF16, tag="wv", name="wv")
            bm_sb = singles.tile([128, 4], F32, tag="bm", name="bm")
            bq_sb = [bm_sb[:, m:m + 1] for m in range(2)]
            bk_sb = [bm_sb[:, 2 + m:3 + m] for m in range(2)]
            bvr_sb = singles.tile([128, EC], F32, tag="bvr", name="bvr")

            # ---- engine warm-up (emitted first on their queues) ----
            nc.vector.memset(dummy, 0.0)
            gate_sb = singles.tile([1, 8], BF16, tag="gate", name="gate")

            # ---- input DMAs: merged descriptors, critical-first ----
            def xq_c(c):
                return (xq_sb[:, c * 2048:(c + 1) * 2048],
                        xq[:, c * 2048:(c + 1) * 2048])

            def xk_c(c):
                return (xk_sb[:, c * 2048:(c + 1) * 2048],
                        xk[:, c * 2048:(c + 1) * 2048])

            def xv_c(c):
                return (xv_sb[:, c * 2048:(c + 1) * 2048],
                        xv[:, c * 2048:(c + 1) * 2048])

            def cview(sb):  # [128, 4c*4t*512] -> [128, c, t, n']
                return sb.rearrange("p (c t n) -> p c t n", c=4, t=DT)

            def wview(sb):  # [128, 4t*EC] -> [128, t, e]
                return sb.rearrange("p (t e) -> p t e", t=DT)

            # wave 1 (ungated): QK projection critical path + V pass-0 needs.
            # Later waves are WAW-gated by tiny pre-writes into the DMA
            # destinations (emission order alone does not survive the tile
            # scheduler). Gates live on the GPSIMD queue, which is idle
            # during the ramp -- on the vector queue they head-of-line
            # blocked the prologue bias adds for ~8us.
            nc.sync.dma_start(wq_sb, wq[:, :])
            nc.sync.dma_start(*xq_c(0))
            nc.scalar.dma_start(bm_sb, bmat[:, :])
            nc.scalar.dma_start(wk_sb, wk[:, :])
            nc.scalar.dma_start(*xk_c(0))
            nc.gpsimd.dma_start(wv_sb, wv[:, :])
            nc.gpsimd.dma_start(*xv_c(0))
            nc.gpsimd.dma_start(bvr_sb, bvr[:, :])
            # ACT table preload, after scalar's DMA issues
            actwarm = singles.tile([1, 8], BF16, tag="actwarm", name="actwarm")
            nc.scalar.activation(
                actwarm, dummy[0:1, 0:8], mybir.ActivationFunctionType.Exp,
                scale=SCALE,
            )
            # wave 2: gated on xk c0 arrival
            nc.gpsimd.tensor_copy(xk_sb[0:1, 2048:2050], xk_sb[0:1, 0:2])
            nc.gpsimd.dma_start(*xk_c(1))
            # wave 3: gated on xv c0 arrival (last wave-1 x piece), issued
            # in consumption order; each chunk DMA WAW-waits its pre-write
            trig = xv_sb[0:1, 0:2]
            for sb, cc, dm in ((xv_sb, 1, xv_c), (xk_sb, 2, xk_c),
                               (xv_sb, 2, xv_c), (xk_sb, 3, xk_c),
                               (xv_sb, 3, xv_c), (xq_sb, 1, xq_c),
                               (xq_sb, 2, xq_c), (xq_sb, 3, xq_c)):
                nc.gpsimd.tensor_copy(
                    sb[0:1, cc * 2048:cc * 2048 + 2], trig)
                nc.gpsimd.dma_start(*dm(cc))

            # ---- working tiles ----
            qt_sb = [qkv.tile([128, N], BF16, tag=f"qt{m}", name=f"qt{m}") for m in range(2)]
            kt_sb = [qkv.tile([128, N], BF16, tag=f"kt{m}", name=f"kt{m}") for m in range(2)]
            # ones column FIRST so den lands at psum partition 0 (custom DVE
            # psum reads drop partition offsets on HW; keep all reads at 0)
            v_sb = [qkv.tile([128, HPC * 65], BF16, tag=f"v{t}", name=f"v{t}") for t in range(NT)]
            for t in range(NT):
                ones_view = v_sb[t].rearrange("p (h c) -> p h c", c=65)[:, :, 0:1]
                nc.vector.memset(ones_view, 1.0)


            with (
                tc.tile_pool(name="proj_ps", bufs=2, space="PSUM") as proj_ps,
                tc.tile_pool(name="st_ps", bufs=2, space="PSUM") as st_ps,
                tc.tile_pool(name="ot_ps", bufs=1, space="PSUM") as ot_ps,
                tc.tile_pool(name="pt_sb", bufs=6) as pt_pool,
                tc.tile_pool(name="pti_sb", bufs=4) as pti_pool,
            ):
                # PE warm-up: dummy matmuls flip HAM to 8/8 during DMA (the
                # DMA engines clock with HAM: ramp bursts measured 113-139
                # GB/s during mixed-clock windows vs 266-281 GB/s at 8/8)
                for i in range(17):
                    ps = proj_ps.tile([128, 512], F32, tag="proj", name="warm")
                    nc.tensor.matmul(ps, lhsT=dummy[:, 0:128], rhs=dummy,
                                     start=True, stop=True)

                # -- emitters --
                def emit_qk_group(dst, w_s, x_s, b_s, hp, nch):
                    ps = proj_ps.tile([128, 512], F32, tag="proj", name="qkp")
                    for t in range(DT):
                        nc.tensor.matmul(
                            ps,
                            lhsT=wview(w_s)[:, t, hp * 128:(hp + 1) * 128],
                            rhs=cview(x_s)[:, nch, t, :],
                            start=(t == 0),
                            stop=(t == DT - 1),
                        )
                    nc.vector.tensor_scalar_add(
                        dst[hp][:, nch * 512:(nch + 1) * 512], ps, b_s[hp]
                    )

                def emit_v_group(t):
                    ps = proj_ps.tile([128, EC], F32, tag="proj", name="vp")
                    for d in range(DT):
                        nc.tensor.matmul(
                            ps,
                            lhsT=cview(xv_sb)[:, t // 4, d,
                                              (t % 4) * 128:(t % 4 + 1) * 128],
                            rhs=wview(wv_sb)[:, d, :],
                            start=(d == 0),
                            stop=(d == DT - 1),
                        )
                    v_view = v_sb[t].rearrange("p (h c) -> p h c", c=65)[:, :, 1:65]
                    nc.vector.tensor_add(
                        v_view,
                        ps.rearrange("p (h c) -> p h c", c=64),
                        bvr_sb.rearrange("p (h c) -> p h c", c=64),
                    )

                def emit_filler(f):
                    if f[0] == "v":
                        emit_v_group(f[1])
                    elif f[0] == "q":
                        emit_qk_group(qt_sb, wq_sb, xq_sb, bq_sb, f[1], f[2])
                    else:
                        emit_qk_group(kt_sb, wk_sb, xk_sb, bk_sb, f[1], f[2])

                def emit_s_pair(hp, icol, j):
                    st = st_ps.tile([128, 1024], F32, tag="st", name="st")
                    for half in range(2):
                        ho = half * 64
                        nc.tensor.matmul(
                            st[:, half * 512:(half + 1) * 512],
                            lhsT=kt_sb[hp][ho:ho + 64, j * 128:(j + 1) * 128],
                            rhs=qt_sb[hp][ho:ho + 64, icol:icol + 512],
                            start=True,
                            stop=True,
                        )
                    return st

                def emit_out_pass(p, ot):
                    # divide OT by its den row (psum partition 0; custom DVE
                    # psum reads only work offset-0 on HW; DVE partition
                    # ranges must be 32-aligned so the multiply covers all 65
                    # rows and the DMA slices off row 0).
                    if p < 7:
                        # one fast copy frees the ot psum bank for the next
                        # pass (WAR on its first AV); the divide chain then
                        # runs off SBUF at leisure: recip(den) -> partition
                        # broadcast (pool) -> one DVE multiply -> one DMA
                        # copy on the scalar engine: DVE queue latency gates
                        # the Schraudolph st releases; ACT has slack here
                        ots = fin_pool.tile([65, 1024], F32, tag="ots", name="ots")
                        nc.scalar.copy(ots, ot)
                        rec = fin_pool.tile([1, 1024], F32, tag="rec", name="rec")
                        nc.vector.reciprocal_approx_fast(rec, ots[0:1, :])
                        bc = fin_pool.tile([65, 1024], F32, tag="bc", name="bc")
                        nc.gpsimd.partition_broadcast(bc, rec)
                        osb = fin_pool.tile([65, 1024], F32, tag="osb", name="osb")
                        nc.vector.tensor_tensor(
                            osb, ots, bc, mybir.AluOpType.mult
                        )
                        eng = nc.sync if p % 2 == 0 else nc.gpsimd
                        eng.dma_start(out[p * 64:(p + 1) * 64, :], osb[1:65, :])
                    else:
                        # last pass: the chain is the kernel tail. Skip the
                        # copy (no next pass -> no WAR) and pipeline two
                        # halves across DVE/pool/DMA to cut exposed latency.
                        for half in range(2):
                            sl = slice(half * 512, (half + 1) * 512)
                            rec = fin_pool.tile([1, 512], F32,
                                                tag=f"rech{half}", name="rec")
                            nc.vector.reciprocal_approx_fast(rec, ot[0:1, sl])
                            bc = fin_pool.tile([65, 512], F32,
                                               tag=f"bch{half}", name="bc")
                            nc.gpsimd.partition_broadcast(bc, rec)
                            osb = fin_pool.tile([65, 512], F32,
                                                tag=f"osbh{half}", name="osb")
                            nc.vector.tensor_tensor(
                                osb, ot[:, sl], bc, mybir.AluOpType.mult
                            )
                            eng = nc.sync if half == 0 else nc.gpsimd
                            eng.dma_start(out[p * 64:(p + 1) * 64, sl],
                                          osb[1:65, :])

                # filler schedule: (pass, iter) -> list of jobs
                fillers = {
                    (0, 0): [("v", 0)], (0, 1): [("k", 0, 1), ("v", 1)],
                    (0, 2): [("v", 2), ("v", 3)], (0, 3): [("v", 4)],
                    (0, 4): [("v", 5), ("v", 6)], (0, 5): [("k", 0, 2)],
                    (0, 6): [("v", 7), ("v", 8)], (0, 7): [("v", 9)],
                    (0, 8): [("v", 10), ("v", 11)], (0, 9): [("k", 0, 3)],
                    (0, 10): [("v", 12), ("v", 13)],
                    (0, 11): [("q", 0, 1)],
                    (0, 12): [("v", 14)], (0, 13): [("v", 15)],
                    (1, 7): [("q", 0, 2)], (1, 13): [("k", 1, 0)],
                    (1, 14): [("k", 1, 1)],
                    (2, 7): [("q", 0, 3)], (2, 13): [("k", 1, 2)],
                    (2, 14): [("k", 1, 3)],
                    (3, 7): [("q", 1, 0)],
                    (4, 7): [("q", 1, 1)],
                    (5, 7): [("q", 1, 2)],
                    (6, 7): [("q", 1, 3)],
                }
                # -- prologue for pass 0 --
                emit_qk_group(qt_sb, wq_sb, xq_sb, bq_sb, 0, 0)
                emit_qk_group(kt_sb, wk_sb, xk_sb, bk_sb, 0, 0)

                # keep the PE (and so HAM) busy through the input-DMA
                # trickle between the prologue projections and the first S
                # pairs; these run in the xk-c0 wait shadow, in the ot psum
                # bank which nothing touches until pass 0's first AV
                for i in range(12):
                    ps = ot_ps.tile([65, 1024], F32, tag="ot", name="warm2")
                    nc.tensor.matmul(ps[:, 0:512], lhsT=dummy[:, 0:65],
                                     rhs=xq_sb[:, 0:512],
                                     start=True, stop=True)

                prologue = [None, None]
                for p in range(8):
                    hp, c = p // 4, p % 4
                    icol = c * 512
                    ha, hb = 2 * hp, 2 * hp + 1
                    sch = SCH_BY_PASS[p]
                    ot = ot_ps.tile([65, 1024], F32, tag="ot", name="ot")
                    sts = [None] * NT
                    pts = [None] * NT

                    if p == 0:
                        sts[0] = emit_s_pair(hp, icol, 0)
                        sts[1] = emit_s_pair(hp, icol, 1)
                    else:
                        sts[0], sts[1] = prologue

                    def emit_av_pair(j):
                        for half, hd in ((0, ha), (1, hb)):
                            nc.tensor.matmul(
                                ot[:, half * 512:(half + 1) * 512],
                                lhsT=v_sb[j][:, hd * 65:(hd + 1) * 65],
                                rhs=pts[j][:, half * 512:(half + 1) * 512],
                                start=(j == 0),
                                stop=(j == NT - 1),
                            )

                    def emit_sch(j):
                        # Schraudolph exp on DVE, one iter ahead of its slot
                        # so the st-buffer WAR never stalls the S pipeline
                        pti = pti_pool.tile([128, 1024], I16, tag="pti", name="pti")
                        nc.vector.tensor_scalar(
                            pti, sts[j], A_SCH, B_SCH,
                            mybir.AluOpType.mult, mybir.AluOpType.add,
                        )
                        pts[j] = pti.bitcast(BF16)

                    if 0 in sch:
                        emit_sch(0)
                    for j in range(NT):
                        if j not in sch:
                            pt = pt_pool.tile([128, 1024], BF16, tag="pt", name="pt")
                            nc.scalar.activation(
                                pt, sts[j], mybir.ActivationFunctionType.Exp,
                                scale=SCALE,
                            )
                            pts[j] = pt
                        if j + 1 in sch:
                            emit_sch(j + 1)
                        if j >= 2:
                            emit_av_pair(j - 2)
                        if j % 2 == 0:
                            for jj in (j + 2, j + 3):
                                if jj < NT:
                                    sts[jj] = emit_s_pair(hp, icol, jj)
                                elif p + 1 < 8:
                                    nhp, nc_ = (p + 1) // 4, (p + 1) % 4
                                    prologue[jj - NT] = emit_s_pair(
                                        nhp, nc_ * 512, jj - NT)
                        for f in fillers.get((p, j), ()):
                            emit_filler(f)

                    emit_av_pair(NT - 2)
                    emit_av_pair(NT - 1)
                    emit_out_pass(p, ot)

    nc.compile()
    return nc


def _get_nc():
    if "nc" not in _cache:
        _cache["nc"] = _build()
    return _cache["nc"]


def _ilv_x(xT):
    # [D, N] -> [128, 4c * 4t * 512n'] with row p = [c][t][n'] interleave
    return np.ascontiguousarray(
        xT.reshape(DT, 128, 4, 512).transpose(1, 2, 0, 3).reshape(128, DT * N)
    ).astype(NP_BF16)


def _ilv_w(w):
    # [D, EC] -> [128, 4t * EC]
    return np.ascontiguousarray(
        w.reshape(DT, 128, EC).transpose(1, 0, 2).reshape(128, DT * EC)
    ).astype(NP_BF16)


def _shard_inputs(q, k, v, Wq, Wk, Wv, bq, bk, bv):
    in_maps = []
    for c in range(8):
        b, g = c // 2, c % 2
        sl = slice(g * EC, (g + 1) * EC)
        in_maps.append({
            "xq": _ilv_x(np.asarray(q)[b].T),
            "xk": _ilv_x(np.asarray(k)[b].T),
            "xv": _ilv_x(np.asarray(v)[b].T),
            "wq": _ilv_w(np.asarray(Wq)[:, sl]),
            "wk": _ilv_w(np.asarray(Wk)[:, sl]),
            "wv": _ilv_w(np.asarray(Wv)[:, sl]),
            "bmat": np.stack([
                np.asarray(bq)[sl][0:128], np.asarray(bq)[sl][128:256],
                np.asarray(bk)[sl][0:128], np.asarray(bk)[sl][128:256],
            ], axis=1).astype(np.float32),
            "bvr": np.ascontiguousarray(
                np.broadcast_to(np.asarray(bv)[sl], (128, EC))
            ).astype(np.float32),
        })
    return in_maps


def kernel(q, k, v, Wq, Wk, Wv, bq, bk, bv, _trace=False):
    nc = _get_nc()
    in_maps = _shard_inputs(q, k, v, Wq, Wk, Wv, bq, bk, bv)
    res = run_bass_kernel_spmd(
        nc, in_maps, core_ids=list(range(8)), trace=_trace
    )
    out = np.empty((B, N, E), np.float32)
    for c in range(8):
        b, g = c // 2, c % 2
        # [8p*64, 1024] -> [hp, cchunk, ch, half, i] -> [N, EC]
        o2 = res.results[c]["out"].reshape(2, 4, 64, 2, 512)
        out[b, :, g * EC:(g + 1) * EC] = (
            o2.transpose(1, 4, 0, 3, 2).reshape(N, EC)
        )
    if _trace:
        _cache["last_exec_time_ns"] = res.exec_time_ns
    return out



# revision 35
# speedup vs baseline: 1.1195x; 1.0189x over previous
"""Multi-head attention TRN2 Bass kernel (v2).

Problem: B=4, N=2048, D=E=512, 8 heads (ch=64).
out = softmax((x_q Wq + bq)(x_k Wk + bk)^T / 8) (x_v Wv + bv), per head.

Sharding (8 cores): core c handles batch b = c//2 and head-group g = c%2
(4 heads = 256 E-columns). Each core is fully independent (no collectives).

v2 changes over the original ACT-paced design:
  - Pass = (head-pair, i-chunk of 512). The two heads of a pair occupy
    SBUF partitions 0-63 / 64-127 of QT/KT, so their S^T matmuls issue as
    back-to-back row-tiled pairs (tile_position (0,0)/(64,0)) that execute
    CONCURRENTLY on the PE (HW-probed: 113 ns/MM vs 215 serial, 1.9x).
  - Part of the exp work moves off the ACT engine onto the DVE as a
    Schraudolph bit-trick: P_bf16bits = int16(rint(A*S + B)), one
    tensor_scalar (fp32 PSUM -> int16 SBUF, round-to-nearest verified on
    HW), bitcast to bf16 for the AV matmul. Host-simulated rel-err with
    this split: ~0.012 (gate 0.02).
  - Inputs are host-interleaved so every chunk DMA is a flat 2D copy with
    4KB-contiguous lines per partition (DMA packets are overhead-bound);
    biases ride one [128,4] tensor. Later waves are WAW-gated behind the
    critical Q/K pieces via tiny pre-writes into their destinations.
  - PE warm-up dummies + early exp-table preload hide the HAM cold clock
    (4/8 = 1.2 GHz) and the 2.7us ACT table load during the input DMA.
  - Output blocks are staged 4-at-a-time in SBUF and written with one DMA
    per (head, 512-chunk): 16 output DMAs instead of 64.

v3: output stays transposed. The AV result OT [ch+1, i] is divided by the
den row in-place ([ch, i] layout) and DMA'd per pass; the host reassembly
transposes blocks (it already reshuffles them). Removes all 64 PE
transposes (~10us of PE), the per-block DVE recip/mul path (~25us of
DVE), and shrinks the post-matmul tail. Division chain per pass:
reciprocal_approx_fast on the den row -> gpsimd partition_broadcast ->
one tensor_tensor multiply.
"""

import numpy as np
import ml_dtypes

import concourse.bacc as bacc
import concourse.mybir as mybir
import concourse.tile as tile
from concourse.bass_utils import run_bass_kernel_spmd

B, N, D, E = 4, 2048, 512, 512
H, CH = 8, 64
HPC = 4              # heads per core
EC = HPC * CH        # 256 E-columns per core
SCALE = 1.0 / 8.0    # 1/sqrt(CH)
NT = N // 128        # 16 j-tiles
DT = D // 128        # 4 d-tiles

SIGMA = 0.055
A_SCH = float(np.float32(128.0 * np.log2(np.e) * SCALE))
B_SCH = float(np.float32(128.0 * (127.0 - SIGMA)))
# Schraudolph j-tiles per pass (ACT exp alone paces both pass 0's ramp and
# the steady passes; host err sim: 4 tiles/pass -> 0.0122, 5 -> 0.0128,
# gate 0.02; j=15 keeps the pass-boundary st WAR off the last ACT)
SCH_BY_PASS = [(2, 5, 9, 12)] + [(2, 5, 9, 12, 15)] * 7

F32 = mybir.dt.float32
BF16 = mybir.dt.bfloat16
I16 = mybir.dt.int16
NP_BF16 = ml_dtypes.bfloat16

_cache = {}


def _build():
    nc = bacc.Bacc("TRN2", target_bir_lowering=False, debug=False)

    # x tensors host-interleaved: row p = [c-major][t-major][n'] so a
    # 512-column chunk is 4KB contiguous per row (DMA packets are
    # overhead-bound, so line size is the bandwidth lever)
    xq = nc.dram_tensor("xq", [128, DT * N], BF16, kind="ExternalInput")
    xk = nc.dram_tensor("xk", [128, DT * N], BF16, kind="ExternalInput")
    xv = nc.dram_tensor("xv", [128, DT * N], BF16, kind="ExternalInput")
    wq = nc.dram_tensor("wq", [128, DT * EC], BF16, kind="ExternalInput")
    wk = nc.dram_tensor("wk", [128, DT * EC], BF16, kind="ExternalInput")
    wv = nc.dram_tensor("wv", [128, DT * EC], BF16, kind="ExternalInput")
    # v-bias broadcast plus the four qk bias columns (bq m0, bq m1, bk m0,
    # bk m1) in ONE f32 tensor: a separate [128, 4] DMA degenerates to
    # 4-16B packets that squat the input DMA queue for ~5us mid-wave-1
    bvr = nc.dram_tensor("bvr", [128, EC + 4], F32, kind="ExternalInput")
    # one [64ch, 1024] block per pass (cols: head-pair x 512 i); host
    # reassembles + transposes
    out = nc.dram_tensor("out", [8 * 64, 1024], F32, kind="ExternalOutput")

    with tile.TileContext(nc) as tc:
        with (
            tc.tile_pool(name="singles", bufs=1) as singles,
            tc.tile_pool(name="qkv", bufs=1) as qkv,
            tc.tile_pool(name="fin", bufs=3) as fin_pool,
        ):
            # ---- SBUF staging ----
            dummy = singles.tile([128, 512], BF16, tag="dummy", name="dummy")
            # flat, chunk-major (c, t, n') so every chunk DMA is a 2D copy
            # with 4KB contiguous per partition (max DMA packet size)
            xq_sb = singles.tile([128, DT * N], BF16, tag="xq", name="xq")
            xk_sb = singles.tile([128, DT * N], BF16, tag="xk", name="xk")
            xv_sb = singles.tile([128, DT * N], BF16, tag="xv", name="xv")
            wq_sb = singles.tile([128, DT * EC], BF16, tag="wq", name="wq")
            wk_sb = singles.tile([128, DT * EC], BF16, tag="wk", name="wk")
            wv_sb = singles.tile([128, DT * EC], BF16, tag="wv", name="wv")
            bvr_sb = singles.tile([128, EC + 4], F32, tag="bvr", name="bvr")
            bq_sb = [bvr_sb[:, EC + m:EC + m + 1] for m in range(2)]
            bk_sb = [bvr_sb[:, EC + 2 + m:EC + 3 + m] for m in range(2)]

            # ---- engine warm-up (emitted first on their queues) ----
            nc.vector.memset(dummy, 0.0)
            gate_sb = singles.tile([1, 8], BF16, tag="gate", name="gate")

            # ---- input DMAs: merged descriptors, critical-first ----
            def xq_c(c):
                return (xq_sb[:, c * 2048:(c + 1) * 2048],
                        xq[:, c * 2048:(c + 1) * 2048])

            def xk_c(c):
                return (xk_sb[:, c * 2048:(c + 1) * 2048],
                        xk[:, c * 2048:(c + 1) * 2048])

            def xv_c(c):
                return (xv_sb[:, c * 2048:(c + 1) * 2048],
                        xv[:, c * 2048:(c + 1) * 2048])

            def cview(sb):  # [128, 4c*4t*512] -> [128, c, t, n']
                return sb.rearrange("p (c t n) -> p c t n", c=4, t=DT)

            def wview(sb):  # [128, 4t*EC] -> [128, t, e]
                return sb.rearrange("p (t e) -> p t e", t=DT)

            # wave 1 (ungated): QK projection critical path + V pass-0 needs.
            # Later waves are WAW-gated by tiny pre-writes into the DMA
            # destinations (emission order alone does not survive the tile
            # scheduler). Gates live on the GPSIMD queue, which is idle
            # during the ramp -- on the vector queue they head-of-line
            # blocked the prologue bias adds for ~8us.
            nc.sync.dma_start(wq_sb, wq[:, :])
            nc.sync.dma_start(*xq_c(0))
            nc.sync.dma_start(bvr_sb, bvr[:, :])
            nc.scalar.dma_start(wk_sb, wk[:, :])
            nc.scalar.dma_start(*xk_c(0))
            nc.gpsimd.dma_start(wv_sb, wv[:, :])
            nc.gpsimd.dma_start(*xv_c(0))
            # ACT table preload, after scalar's DMA issues
            actwarm = singles.tile([1, 8], BF16, tag="actwarm", name="actwarm")
            nc.scalar.activation(
                actwarm, dummy[0:1, 0:8], mybir.ActivationFunctionType.Exp,
                scale=SCALE,
            )
            # wave 2: gated on xk c0 arrival
            nc.gpsimd.tensor_copy(xk_sb[0:1, 2048:2050], xk_sb[0:1, 0:2])
            nc.gpsimd.dma_start(*xk_c(1))
            # wave 3: gated on xv c0 arrival (last wave-1 x piece), issued
            # in consumption order; each chunk DMA WAW-waits its pre-write
            trig = xv_sb[0:1, 0:2]
            for sb, cc, dm in ((xv_sb, 1, xv_c), (xk_sb, 2, xk_c),
                               (xv_sb, 2, xv_c), (xk_sb, 3, xk_c),
                               (xv_sb, 3, xv_c), (xq_sb, 1, xq_c),
                               (xq_sb, 2, xq_c), (xq_sb, 3, xq_c)):
                nc.gpsimd.tensor_copy(
                    sb[0:1, cc * 2048:cc * 2048 + 2], trig)
                nc.gpsimd.dma_start(*dm(cc))

            # ---- working tiles ----
            qt_sb = [qkv.tile([128, N], BF16, tag=f"qt{m}", name=f"qt{m}") for m in range(2)]
            kt_sb = [qkv.tile([128, N], BF16, tag=f"kt{m}", name=f"kt{m}") for m in range(2)]
            # ones column FIRST so den lands at psum partition 0 (custom DVE
            # psum reads drop partition offsets on HW; keep all reads at 0)
            v_sb = [qkv.tile([128, HPC * 65], BF16, tag=f"v{t}", name=f"v{t}") for t in range(NT)]
            for t in range(NT):
                ones_view = v_sb[t].rearrange("p (h c) -> p h c", c=65)[:, :, 0:1]
                nc.vector.memset(ones_view, 1.0)


            with (
                tc.tile_pool(name="proj_ps", bufs=2, space="PSUM") as proj_ps,
                tc.tile_pool(name="st_ps", bufs=2, space="PSUM") as st_ps,
                tc.tile_pool(name="ot_ps", bufs=1, space="PSUM") as ot_ps,
                tc.tile_pool(name="pt_sb", bufs=6) as pt_pool,
                tc.tile_pool(name="pti_sb", bufs=4) as pti_pool,
            ):
                # PE warm-up: dummy matmuls flip HAM to 8/8 during DMA (the
                # DMA engines clock with HAM: ramp bursts measured 113-139
                # GB/s during mixed-clock windows vs 266-281 GB/s at 8/8)
                for i in range(17):
                    ps = proj_ps.tile([128, 512], F32, tag="proj", name="warm")
                    nc.tensor.matmul(ps, lhsT=dummy[:, 0:128], rhs=dummy,
                                     start=True, stop=True)

                # -- emitters --
                def emit_qk_group(dst, w_s, x_s, b_s, hp, nch):
                    ps = proj_ps.tile([128, 512], F32, tag="proj", name="qkp")
                    for t in range(DT):
                        nc.tensor.matmul(
                            ps,
                            lhsT=wview(w_s)[:, t, hp * 128:(hp + 1) * 128],
                            rhs=cview(x_s)[:, nch, t, :],
                            start=(t == 0),
                            stop=(t == DT - 1),
                        )
                    nc.vector.tensor_scalar_add(
                        dst[hp][:, nch * 512:(nch + 1) * 512], ps, b_s[hp]
                    )

                def emit_v_group(t):
                    ps = proj_ps.tile([128, EC], F32, tag="proj", name="vp")
                    for d in range(DT):
                        nc.tensor.matmul(
                            ps,
                            lhsT=cview(xv_sb)[:, t // 4, d,
                                              (t % 4) * 128:(t % 4 + 1) * 128],
                            rhs=wview(wv_sb)[:, d, :],
                            start=(d == 0),
                            stop=(d == DT - 1),
                        )
                    v_view = v_sb[t].rearrange("p (h c) -> p h c", c=65)[:, :, 1:65]
                    nc.vector.tensor_add(
                        v_view,
                        ps.rearrange("p (h c) -> p h c", c=64),
                        bvr_sb[:, 0:EC].rearrange("p (h c) -> p h c", c=64),
                    )

                def emit_filler(f):
                    if f[0] == "v":
                        emit_v_group(f[1])
                    elif f[0] == "q":
                        emit_qk_group(qt_sb, wq_sb, xq_sb, bq_sb, f[1], f[2])
                    else:
                        emit_qk_group(kt_sb, wk_sb, xk_sb, bk_sb, f[1], f[2])

                def emit_s_pair(hp, icol, j):
                    st = st_ps.tile([128, 1024], F32, tag="st", name="st")
                    for half in range(2):
                        ho = half * 64
                        nc.tensor.matmul(
                            st[:, half * 512:(half + 1) * 512],
                            lhsT=kt_sb[hp][ho:ho + 64, j * 128:(j + 1) * 128],
                            rhs=qt_sb[hp][ho:ho + 64, icol:icol + 512],
                            start=True,
                            stop=True,
                        )
                    return st

                def emit_out_pass(p, ot):
                    # divide OT by its den row (psum partition 0; custom DVE
                    # psum reads only work offset-0 on HW; DVE partition
                    # ranges must be 32-aligned so the multiply covers all 65
                    # rows and the DMA slices off row 0).
                    if p < 7:
                        # one fast copy frees the ot psum bank for the next
                        # pass (WAR on its first AV); the divide chain then
                        # runs off SBUF at leisure: recip(den) -> partition
                        # broadcast (pool) -> one DVE multiply -> one DMA
                        # copy on the scalar engine: DVE queue latency gates
                        # the Schraudolph st releases; ACT has slack here
                        ots = fin_pool.tile([65, 1024], F32, tag="ots", name="ots")
                        nc.scalar.copy(ots, ot)
                        rec = fin_pool.tile([1, 1024], F32, tag="rec", name="rec")
                        nc.vector.reciprocal_approx_fast(rec, ots[0:1, :])
                        bc = fin_pool.tile([65, 1024], F32, tag="bc", name="bc")
                        nc.gpsimd.partition_broadcast(bc, rec)
                        osb = fin_pool.tile([65, 1024], F32, tag="osb", name="osb")
                        nc.vector.tensor_tensor(
                            osb, ots, bc, mybir.AluOpType.mult
                        )
                        eng = nc.sync if p % 2 == 0 else nc.gpsimd
                        eng.dma_start(out[p * 64:(p + 1) * 64, :], osb[1:65, :])
                    else:
                        # last pass: the chain is the kernel tail. Skip the
                        # copy (no next pass -> no WAR) and pipeline two
                        # halves across DVE/pool/DMA to cut exposed latency.
                        for half in range(2):
                            sl = slice(half * 512, (half + 1) * 512)
                            rec = fin_pool.tile([1, 512], F32,
                                                tag=f"rech{half}", name="rec")
                            nc.vector.reciprocal_approx_fast(rec, ot[0:1, sl])
                            bc = fin_pool.tile([65, 512], F32,
                                               tag=f"bch{half}", name="bc")
                            nc.gpsimd.partition_broadcast(bc, rec)
                            osb = fin_pool.tile([65, 512], F32,
                                                tag=f"osbh{half}", name="osb")
                            nc.vector.tensor_tensor(
                                osb, ot[:, sl], bc, mybir.AluOpType.mult
                            )
                            eng = nc.sync if half == 0 else nc.gpsimd
                            eng.dma_start(out[p * 64:(p + 1) * 64, sl],
                                          osb[1:65, :])

                # filler schedule: (pass, iter) -> list of jobs
                fillers = {
                    (0, 0): [("v", 0)], (0, 1): [("k", 0, 1), ("v", 1)],
                    (0, 2): [("v", 2), ("v", 3)], (0, 3): [("v", 4)],
                    (0, 4): [("v", 5), ("v", 6)], (0, 5): [("k", 0, 2)],
                    (0, 6): [("v", 7), ("v", 8)], (0, 7): [("v", 9)],
                    (0, 8): [("v", 10), ("v", 11)], (0, 9): [("k", 0, 3)],
                    (0, 10): [("v", 12), ("v", 13)],
                    (0, 11): [("q", 0, 1)],
                    (0, 12): [("v", 14)], (0, 13): [("v", 15)],
                    (1, 7): [("q", 0, 2)], (1, 13): [("k", 1, 0)],
                    (1, 14): [("k", 1, 1)],
                    (2, 7): [("q", 0, 3)], (2, 13): [("k", 1, 2)],
                    (2, 14): [("k", 1, 3)],
                    (3, 7): [("q", 1, 0)],
                    (4, 7): [("q", 1, 1)],
                    (5, 7): [("q", 1, 2)],
                    (6, 7): [("q", 1, 3)],
                }
                # -- prologue for pass 0 --
                emit_qk_group(qt_sb, wq_sb, xq_sb, bq_sb, 0, 0)
                emit_qk_group(kt_sb, wk_sb, xk_sb, bk_sb, 0, 0)

                # keep the PE (and so HAM) busy through the input-DMA
                # trickle between the prologue projections and the first S
                # pairs; these run in the xk-c0 wait shadow, in the ot psum
                # bank which nothing touches until pass 0's first AV
                for i in range(12):
                    ps = ot_ps.tile([65, 1024], F32, tag="ot", name="warm2")
                    nc.tensor.matmul(ps[:, 0:512], lhsT=dummy[:, 0:65],
                                     rhs=xq_sb[:, 0:512],
                                     start=True, stop=True)

                prologue = [None, None]
                for p in range(8):
                    hp, c = p // 4, p % 4
                    icol = c * 512
                    ha, hb = 2 * hp, 2 * hp + 1
                    sch = SCH_BY_PASS[p]
                    ot = ot_ps.tile([65, 1024], F32, tag="ot", name="ot")
                    sts = [None] * NT
                    pts = [None] * NT

                    if p == 0:
                        sts[0] = emit_s_pair(hp, icol, 0)
                        sts[1] = emit_s_pair(hp, icol, 1)
                    else:
                        sts[0], sts[1] = prologue

                    def emit_av_pair(j):
                        for half, hd in ((0, ha), (1, hb)):
                            nc.tensor.matmul(
                                ot[:, half * 512:(half + 1) * 512],
                                lhsT=v_sb[j][:, hd * 65:(hd + 1) * 65],
                                rhs=pts[j][:, half * 512:(half + 1) * 512],
                                start=(j == 0),
                                stop=(j == NT - 1),
                            )

                    def emit_sch(j):
                        # Schraudolph exp on DVE, one iter ahead of its slot
                        # so the st-buffer WAR never stalls the S pipeline
                        pti = pti_pool.tile([128, 1024], I16, tag="pti", name="pti")
                        nc.vector.tensor_scalar(
                            pti, sts[j], A_SCH, B_SCH,
                            mybir.AluOpType.mult, mybir.AluOpType.add,
                        )
                        pts[j] = pti.bitcast(BF16)

                    if 0 in sch:
                        emit_sch(0)
                    for j in range(NT):
                        if j not in sch:
                            pt = pt_pool.tile([128, 1024], BF16, tag="pt", name="pt")
                            nc.scalar.activation(
                                pt, sts[j], mybir.ActivationFunctionType.Exp,
                                scale=SCALE,
                            )
                            pts[j] = pt
                        if j + 1 in sch:
                            emit_sch(j + 1)
                        if j >= 2:
                            emit_av_pair(j - 2)
                        if j % 2 == 0:
                            for jj in (j + 2, j + 3):
                                if jj < NT:
                                    sts[jj] = emit_s_pair(hp, icol, jj)
                                elif p + 1 < 8:
                                    nhp, nc_ = (p + 1) // 4, (p + 1) % 4
                                    prologue[jj - NT] = emit_s_pair(
                                        nhp, nc_ * 512, jj - NT)
                        for f in fillers.get((p, j), ()):
                            emit_filler(f)

                    emit_av_pair(NT - 2)
                    emit_av_pair(NT - 1)
                    emit_out_pass(p, ot)

    nc.compile()
    return nc


def _get_nc():
    if "nc" not in _cache:
        _cache["nc"] = _build()
    return _cache["nc"]


def _ilv_x(xT):
    # [D, N] -> [128, 4c * 4t * 512n'] with row p = [c][t][n'] interleave
    return np.ascontiguousarray(
        xT.reshape(DT, 128, 4, 512).transpose(1, 2, 0, 3).reshape(128, DT * N)
    ).astype(NP_BF16)


def _ilv_w(w):
    # [D, EC] -> [128, 4t * EC]
    return np.ascontiguousarray(
        w.reshape(DT, 128, EC).transpose(1, 0, 2).reshape(128, DT * EC)
    ).astype(NP_BF16)


def _shard_inputs(q, k, v, Wq, Wk, Wv, bq, bk, bv):
    in_maps = []
    for c in range(8):
        b, g = c // 2, c % 2
        sl = slice(g * EC, (g + 1) * EC)
        in_maps.append({
            "xq": _ilv_x(np.asarray(q)[b].T),
            "xk": _ilv_x(np.asarray(k)[b].T),
            "xv": _ilv_x(np.asarray(v)[b].T),
            "wq": _ilv_w(np.asarray(Wq)[:, sl]),
            "wk": _ilv_w(np.asarray(Wk)[:, sl]),
            "wv": _ilv_w(np.asarray(Wv)[:, sl]),
            "bvr": np.concatenate([
                np.broadcast_to(np.asarray(bv)[sl], (128, EC)),
                np.stack([
                    np.asarray(bq)[sl][0:128], np.asarray(bq)[sl][128:256],
                    np.asarray(bk)[sl][0:128], np.asarray(bk)[sl][128:256],
                ], axis=1),
            ], axis=1).astype(np.float32),
        })
    return in_maps


def kernel(q, k, v, Wq, Wk, Wv, bq, bk, bv, _trace=False):
    nc = _get_nc()
    in_maps = _shard_inputs(q, k, v, Wq, Wk, Wv, bq, bk, bv)
    res = run_bass_kernel_spmd(
        nc, in_maps, core_ids=list(range(8)), trace=_trace
    )
    out = np.empty((B, N, E), np.float32)
    for c in range(8):
        b, g = c // 2, c % 2
        # [8p*64, 1024] -> [hp, cchunk, ch, half, i] -> [N, EC]
        o2 = res.results[c]["out"].reshape(2, 4, 64, 2, 512)
        out[b, :, g * EC:(g + 1) * EC] = (
            o2.transpose(1, 4, 0, 3, 2).reshape(N, EC)
        )
    if _trace:
        _cache["last_exec_time_ns"] = res.exec_time_ns
    return out

